# revision 23
# baseline (speedup 1.0000x reference)
"""Trainium2 Bass kernel for single-query ragged attention.

reference math (per batch b, L = seq_len[b]):
  q = leaky(dec @ Wq + bq)                   [KQ]
  k = leaky(enc[:L] @ Wk + bk)               [L, KQ]
  v = leaky(enc[:L] @ Wv + bv)               [L, VD]
  e = k @ q                                  [L]
  s = softmax(e) (masked to L, renormalized) [T] (zeros beyond L)
  ctx = s[:L] @ v                            [VD]

Strategy: data-parallel over batch across 8 NeuronCores, 8 batch "slots"
per core. Ragged: only ceil(L/128) row-tiles per batch are computed; the
kernel is compile-time specialized to the actual seq_len profile (slot j
runs max-over-cores tiles; shorter batches are zero-padded and masked).
Host pre-transposes encoder shards to [ENC, L] bf16 so the contraction
dim lands on SBUF partitions. Compute: bf16 matmuls w/ f32 PSUM
accumulation, f32 softmax statistics.
"""

import math

import ml_dtypes
import numpy as np

import concourse.bass as bass
import concourse.tile as tile
from concourse import bacc, mybir

B, T, ENC = 64, 2048, 512
KQ, VD = 256, 256
KV = KQ + VD
NEG_SLOPE = 0.2
N_CORES = 8
SLOTS = 8          # batches per core
PT = 128           # partition tile (rows of t per tile)
ECH = ENC // 128   # 4 contraction chunks
CHUNK = 4          # t-tiles per DMA

F32 = mybir.dt.float32
BF16 = mybir.dt.bfloat16
AF = mybir.ActivationFunctionType
OP = mybir.AluOpType

LEAKY_IMPL = "act"  # "act" = ScalarE Lrelu; "dve" = VectorE mul+max fallback

_CACHE = {}


# ---------------------------------------------------------------- host prep

def _assign(seq_len):
    """Distribute 64 batches into 8 cores x 8 slots, balancing tile counts.

    Returns (batch_of[core][slot], slot_tiles[slot]) where slot_tiles is the
    per-slot tile count shared by all cores (max over cores, for SPMD)."""
    ntiles = [max(1, math.ceil(int(l) / PT)) for l in seq_len]
    order = sorted(range(B), key=lambda b: -ntiles[b])
    groups = [[] for _ in range(N_CORES)]
    # snake distribution: balanced totals AND similar sorted profiles
    for r in range(SLOTS):
        idxs = order[r * N_CORES:(r + 1) * N_CORES]
        if r % 2 == 1:
            idxs = idxs[::-1]
        for g in range(N_CORES):
            groups[g].append(idxs[g])
    for g in range(N_CORES):
        groups[g].sort(key=lambda b: -ntiles[b])
    slot_tiles = [max(ntiles[groups[g][j]] for g in range(N_CORES))
                  for j in range(SLOTS)]
    return groups, slot_tiles


def _prepare(decoder_feat, encoder_feat, Wq, bq, Wk, bk, Wv, bv, seq_len):
    dec = np.ascontiguousarray(decoder_feat, dtype=np.float32)
    enc = np.ascontiguousarray(encoder_feat, dtype=np.float32)
    L = np.clip(np.asarray(seq_len).astype(np.int64), 1, T)
    assert dec.shape == (B, ENC) and enc.shape == (B, T, ENC)

    groups, slot_tiles = _assign(L)
    nt_total = sum(slot_tiles)
    pad = (-nt_total) % CHUNK
    slot_tiles = list(slot_tiles)
    slot_tiles[-1] += pad
    nt_total += pad
    slot_off = np.cumsum([0] + slot_tiles)[:-1]

    bias_kv = np.concatenate([np.asarray(bk, np.float32).reshape(KQ),
                              np.asarray(bv, np.float32).reshape(VD)])
    bq = np.asarray(bq, np.float32).reshape(KQ)
    has_bias_kv = bool(np.any(bias_kv))
    has_bias_q = bool(np.any(bq))

    wkv = np.concatenate([np.asarray(Wk, np.float32),
                          np.asarray(Wv, np.float32)], axis=1)  # [ENC, 512]
    wkv16 = wkv.astype(ml_dtypes.bfloat16)
    wq16 = np.asarray(Wq, np.float32).astype(ml_dtypes.bfloat16)
    sel = np.zeros((SLOTS, SLOTS * 128), np.float32)
    for j in range(SLOTS):
        sel[j, j * 128:(j + 1) * 128] = 1.0
    sel16 = sel.astype(ml_dtypes.bfloat16)
    ident = np.eye(128, dtype=np.float32)

    in_maps = []
    meta = []  # per core: list of (batch, L)
    for g in range(N_CORES):
        encT = np.zeros((ENC, nt_total * PT), np.float32)
        masks = np.zeros((PT, nt_total), np.float32)
        decT = np.zeros((ENC, SLOTS), np.float32)
        core_meta = []
        for j in range(SLOTS):
            b = groups[g][j]
            l = int(L[b])
            off = int(slot_off[j]) * PT
            encT[:, off:off + l] = enc[b, :l].T
            decT[:, j] = dec[b]
            for i in range(slot_tiles[j]):
                lo = i * PT
                valid = min(max(l - lo, 0), PT)
                masks[:valid, int(slot_off[j]) + i] = 1.0
            core_meta.append((b, l))
        meta.append(core_meta)
        in_maps.append({
            "encT": encT.astype(ml_dtypes.bfloat16),
            "decT": decT.astype(ml_dtypes.bfloat16),
            "wkv": wkv16,
            "wq": wq16,
            "sel": sel16,
            "ident": ident,
            "masks": masks.astype(ml_dtypes.bfloat16),
            "bias_kv": np.broadcast_to(bias_kv, (PT, KV)).copy(),
            "bq8": np.broadcast_to(bq, (SLOTS, KQ)).copy(),
        })

    key = (tuple(slot_tiles), has_bias_kv, has_bias_q)
    if key not in _CACHE:
        _CACHE[key] = _build(slot_tiles, has_bias_kv, has_bias_q)
    return _CACHE[key], in_maps, meta


# ---------------------------------------------------------------- device

def _build(slot_tiles, has_bias_kv, has_bias_q, leaky_impl=None, debug_stage=99):
    leaky_impl = leaky_impl or LEAKY_IMPL
    nt_total = sum(slot_tiles)
    slot_off = np.cumsum([0] + list(slot_tiles))[:-1]
    nc = bacc.Bacc("TRN2", target_bir_lowering=False, debug=False)

    encT_p = nc.declare_dram_parameter("encT", [ENC, nt_total * PT], BF16, isOutput=False)
    decT_p = nc.declare_dram_parameter("decT", [ENC, SLOTS], BF16, isOutput=False)
    wkv_p = nc.declare_dram_parameter("wkv", [ENC, KV], BF16, isOutput=False)
    wq_p = nc.declare_dram_parameter("wq", [ENC, KQ], BF16, isOutput=False)
    sel_p = nc.declare_dram_parameter("sel", [SLOTS, SLOTS * 128], BF16, isOutput=False)
    ident_p = nc.declare_dram_parameter("ident", [128, 128], F32, isOutput=False)
    masks_p = nc.declare_dram_parameter("masks", [PT, nt_total], BF16, isOutput=False)
    bias_kv_p = nc.declare_dram_parameter("bias_kv", [PT, KV], F32, isOutput=False)
    bq8_p = nc.declare_dram_parameter("bq8", [SLOTS, KQ], F32, isOutput=False)
    score_p = nc.declare_dram_parameter("score_out", [SLOTS, T], F32, isOutput=True)
    ctx_p = nc.declare_dram_parameter("ctx_out", [SLOTS, VD], F32, isOutput=True)

    with tile.TileContext(nc) as tc:
        with (
            tc.tile_pool(name="const", bufs=1) as cpool,
            tc.tile_pool(name="enc", bufs=4) as epool,
            tc.tile_pool(name="kv", bufs=nt_total) as kvpool,
            tc.tile_pool(name="small", bufs=2) as spool,
            tc.tile_pool(name="ps_kv", bufs=3, space="PSUM") as ps_kv,
            tc.tile_pool(name="ps_misc", bufs=1, space="PSUM") as ps_misc,
            tc.tile_pool(name="ps_ctx", bufs=2, space="PSUM") as ps_ctx,
            tc.tile_pool(name="ps_tp", bufs=1, space="PSUM") as ps_tp,
            tc.tile_pool(name="ps_tiny", bufs=1, space="PSUM") as ps_tiny,
        ):
            # ---- constants
            wkv_sb = cpool.tile([128, ECH, KV], BF16)
            nc.sync.dma_start(wkv_sb[:], wkv_p[:].rearrange("(c p) n -> p c n", p=128))
            wq_sb = cpool.tile([128, ECH, KQ], BF16)
            nc.sync.dma_start(wq_sb[:], wq_p[:].rearrange("(c p) n -> p c n", p=128))
            decT_sb = cpool.tile([128, ECH, SLOTS], BF16)
            nc.sync.dma_start(decT_sb[:], decT_p[:].rearrange("(c p) n -> p c n", p=128))
            sel_sb = cpool.tile([SLOTS, SLOTS * 128], BF16)
            nc.sync.dma_start(sel_sb[:], sel_p[:])
            ident_sb = cpool.tile([128, 128], F32)
            nc.sync.dma_start(ident_sb[:], ident_p[:])
            masks_sb = cpool.tile([PT, nt_total], BF16)
            nc.sync.dma_start(masks_sb[:], masks_p[:])
            if has_bias_kv:
                bias_kv_sb = cpool.tile([PT, KV], F32)
                nc.sync.dma_start(bias_kv_sb[:], bias_kv_p[:])
            if has_bias_q:
                bq8_sb = cpool.tile([SLOTS, KQ], F32)
                nc.sync.dma_start(bq8_sb[:], bq8_p[:])
            ones_col = cpool.tile([128, 1], F32)
            nc.vector.memset(ones_col[:], 1.0)
            ones_row = cpool.tile([1, 128], F32)
            nc.vector.memset(ones_row[:], 1.0)

            def leaky(out_ap, in_ap, tmp_pool, tmp_shape, tmp_tag):
                if leaky_impl == "act":
                    nc.scalar.activation(out_ap, in_ap, AF.Prelu, alpha=NEG_SLOPE)
                else:
                    tmp = tmp_pool.tile(tmp_shape, F32, tag=tmp_tag)
                    nc.vector.tensor_scalar_mul(tmp[:], in_ap, NEG_SLOPE)
                    nc.vector.tensor_tensor(out_ap, in_ap, tmp[:], OP.max)

            # ---- q projection: q[slot, kq] then per-slot broadcast
            q_ps = ps_misc.tile([SLOTS, KQ], F32, tag="qmisc")
            if has_bias_q:
                nc.vector.tensor_copy(q_ps[:], bq8_sb[:])
            for c in range(ECH):
                nc.tensor.matmul(q_ps[:], decT_sb[:, c, :], wq_sb[:, c, :],
                                 start=(c == 0 and not has_bias_q), stop=(c == ECH - 1))
            q_sb = cpool.tile([SLOTS, KQ], BF16)
            leaky(q_sb[:], q_ps[:], spool, [SLOTS, KQ], "qleak")

            qb_all = cpool.tile([128, SLOTS * KQ], BF16)
            for j in range(SLOTS):
                qb_ps = ps_misc.tile([128, KQ], F32, tag="qmisc")
                nc.tensor.matmul(qb_ps[:], sel_sb[:, j * 128:(j + 1) * 128], q_sb[:],
                                 start=True, stop=True)
                nc.vector.tensor_copy(qb_all[:, j * KQ:(j + 1) * KQ], qb_ps[:])

            if debug_stage == 1:  # dump first qb slot column block and stop
                dump = cpool.tile([128, 128], F32)
                nc.vector.tensor_copy(dump[:], qb_all[:, 0:128])
                nc.sync.dma_start(
                    score_p[:].rearrange("s t -> (s t)")
                    .rearrange("(p x) -> p x", p=128)[:, 0:128], dump[:])

            # ---- phase 1: K/V projections + energies
            # sub-stages for debug: 21=dma only, 22=+matmul, 23=+leaky, 24/2+=full
            energy_all = None
            kv_tiles = []
            if debug_stage >= 2 or debug_stage in (21, 22, 23):
                sub = debug_stage if debug_stage in (21, 22, 23) else 99
                energy_all = cpool.tile([128, nt_total], F32)
                junk = cpool.tile([128, KQ], F32)
                enc_ch = None
                for j in range(SLOTS):
                    for i in range(slot_tiles[j]):
                        g = int(slot_off[j]) + i
                        if g % CHUNK == 0:
                            enc_ch = epool.tile([128, ECH, CHUNK * PT], BF16, tag="ench")
                            nc.sync.dma_start(
                                enc_ch[:],
                                encT_p[:].rearrange("(c p) t -> p c t", p=128)
                                [:, :, g * PT:(g + CHUNK) * PT])
                        t0 = (g % CHUNK) * PT
                        if sub <= 21:
                            continue
                        kv_ps = ps_kv.tile([128, KV], F32, tag="kvps")
                        if has_bias_kv:
                            nc.vector.tensor_copy(kv_ps[:], bias_kv_sb[:])
                        for c in range(ECH):
                            nc.tensor.matmul(kv_ps[:], enc_ch[:, c, t0:t0 + PT],
                                             wkv_sb[:, c, :],
                                             start=(c == 0 and not has_bias_kv),
                                             stop=(c == ECH - 1))
                        kv_sb = kvpool.tile([128, KV], BF16, tag="kv")
                        if sub <= 22:
                            nc.vector.tensor_copy(kv_sb[:], kv_ps[:])
                        else:
                            leaky(kv_sb[:], kv_ps[:], spool, [128, KV], "kvleak")
                        kv_tiles.append(kv_sb)
                        if sub <= 23:
                            continue
                        nc.vector.tensor_mul(junk[:], kv_sb[:, 0:KQ],
                                             qb_all[:, j * KQ:(j + 1) * KQ])
                        nc.vector.tensor_reduce(energy_all[:, g:g + 1], junk[:],
                                                mybir.AxisListType.X, OP.add)

            if debug_stage == 2:  # dump energies
                nc.sync.dma_start(
                    score_p[:].rearrange("s t -> (s t)")
                    .rearrange("(p x) -> p x", p=128)[:, 0:min(nt_total, 128)],
                    energy_all[:, 0:min(nt_total, 128)])

            # ---- phase 2: softmax + context + outputs
            if debug_stage >= 3 and not (21 <= debug_stage <= 23):
                E_all = cpool.tile([128, nt_total], BF16)
                nc.scalar.activation(E_all[:], energy_all[:], AF.Exp)
                Em_all = cpool.tile([128, nt_total], BF16)
                ctx_all = cpool.tile([1, SLOTS * VD], F32)
                for j in range(SLOTS):
                    o, n = int(slot_off[j]), slot_tiles[j]
                    rowsum = spool.tile([128, 1], F32, tag="rowsum")
                    nc.vector.tensor_mul(Em_all[:, o:o + n], E_all[:, o:o + n],
                                         masks_sb[:, o:o + n])
                    nc.vector.tensor_reduce(rowsum[:], Em_all[:, o:o + n],
                                            mybir.AxisListType.X, OP.add)
                    s_ps = ps_tiny.tile([1, 1], F32, tag="tiny")
                    nc.tensor.matmul(s_ps[:], rowsum[:], ones_col[:],
                                     start=True, stop=True)
                    recip = spool.tile([1, 1], F32, tag="recip")
                    nc.vector.reciprocal(recip[:], s_ps[:])
                    bc_ps = ps_tiny.tile([128, 1], F32, tag="tiny")
                    nc.tensor.matmul(bc_ps[:], ones_row[:], recip[:],
                                     start=True, stop=True)
                    recip_col = spool.tile([128, 1], F32, tag="recipcol")
                    nc.vector.tensor_copy(recip_col[:], bc_ps[:])

                    if debug_stage >= 4:
                        # context: accumulate E_masked[:,g] @ v_g, scale by 1/S
                        ctx_ps = ps_ctx.tile([1, VD], F32, tag="ctxps")
                        for i in range(n):
                            g = o + i
                            nc.tensor.matmul(ctx_ps[:], Em_all[:, g:g + 1],
                                             kv_tiles[g][:, KQ:KV],
                                             start=(i == 0), stop=(i == n - 1))
                        nc.vector.tensor_scalar_mul(ctx_all[:, j * VD:(j + 1) * VD],
                                                    ctx_ps[:], recip[:])

                    if debug_stage >= 5:
                        # score: E_masked/S, transposed out to [tiles, 128]
                        nw = min(n, T // PT)
                        score_sb = spool.tile([128, nw], F32, tag="score")
                        nc.vector.tensor_scalar_mul(score_sb[:], Em_all[:, o:o + nw],
                                                    recip_col[:])
                        tp_ps = ps_tp.tile([nw, 128], F32, tag="tp")
                        nc.tensor.transpose(tp_ps[:], score_sb[:], ident_sb[:])
                        scoreT_sb = spool.tile([nw, 128], F32, tag="scoreT")
                        nc.scalar.copy(scoreT_sb[:], tp_ps[:])
                        nc.sync.dma_start(
                            score_p[:].rearrange("s (a b) -> s a b", b=PT)[j, 0:nw, :],
                            scoreT_sb[:])
                if debug_stage >= 4:
                    nc.sync.dma_start(
                        ctx_p[:].rearrange("s v -> (s v)")[None, :], ctx_all[:])

    nc.compile()
    return nc


# ---------------------------------------------------------------- entry

def kernel(decoder_feat, encoder_feat, Wq, bq, Wk, bk, Wv, bv, seq_len):
    from concourse.bass_utils import run_bass_kernel_spmd

    nc, in_maps, meta = _prepare(decoder_feat, encoder_feat, Wq, bq,
                                 Wk, bk, Wv, bv, seq_len)
    res = run_bass_kernel_spmd(nc, in_maps, list(range(N_CORES)))
    return _gather(res.results, meta)


def _gather(results, meta):
    score = np.zeros((B, T), np.float32)
    ctx = np.zeros((B, VD), np.float32)
    for g in range(N_CORES):
        sc = np.asarray(results[g]["score_out"], np.float32)
        cx = np.asarray(results[g]["ctx_out"], np.float32)
        for j, (b, l) in enumerate(meta[g]):
            score[b, :l] = sc[j, :l]
            ctx[b] = cx[j]
    return score, ctx


# revision 42
# speedup vs baseline: 1.1613x; 1.1613x over previous
"""Trainium2 Bass kernel for single-query ragged attention.

reference math (per batch b, L = seq_len[b]):
  q = leaky(dec @ Wq + bq)                   [KQ]
  k = leaky(enc[:L] @ Wk + bk)               [L, KQ]
  v = leaky(enc[:L] @ Wv + bv)               [L, VD]
  e = k @ q                                  [L]
  s = softmax(e) (masked to L, renormalized) [T] (zeros beyond L)
  ctx = s[:L] @ v                            [VD]

Strategy: data-parallel over batch across 8 NeuronCores, 8 batch "slots"
per core. Ragged: only ceil(L/128) row-tiles per batch are computed; the
kernel is compile-time specialized to the actual seq_len profile (slot j
runs max-over-cores tiles; shorter batches are zero-padded and masked).
Host pre-transposes encoder shards to [ENC, L] bf16 so the contraction
dim lands on SBUF partitions. Compute: bf16 matmuls w/ f32 PSUM
accumulation, f32 softmax statistics.

Engine budget per tile (128 t-rows): PE 4x MM[128x128x512] (~213ns ea),
ACT 1/2 Prelu[128,1024] (~500ns), GpSimd product mul[128,256],
DVE per-slot batched reduce. Energy/softmax stats in f32.
"""

import math

import ml_dtypes
import numpy as np

import concourse.bass as bass
import concourse.tile as tile
from concourse import bacc, mybir

B, T, ENC = 64, 2048, 512
KQ, VD = 256, 256
KV = KQ + VD
NEG_SLOPE = 0.2
N_CORES = 8
SLOTS = 8          # batches per core
PT = 128           # partition tile (rows of t per tile)
ECH = ENC // 128   # 4 contraction chunks
CHUNK = 8          # t-tiles per DMA

F32 = mybir.dt.float32
BF16 = mybir.dt.bfloat16
AF = mybir.ActivationFunctionType
OP = mybir.AluOpType
AX = mybir.AxisListType

LEAKY_IMPL = "act"       # "act" = ScalarE Prelu; "dve" = VectorE mul+max
MUL_ENGINE = "gpsimd"    # engine for energy product mul: "gpsimd" | "vector"

_CACHE = {}


# ---------------------------------------------------------------- host prep

def _assign(seq_len):
    """Distribute 64 batches into 8 cores x 8 slots minimizing the padded
    tile total sum_j max_g(ntiles). Snake seed + swap refinement."""
    ntiles = [max(1, math.ceil(int(l) / PT)) for l in seq_len]
    order = sorted(range(B), key=lambda b: -ntiles[b])
    groups = [[] for _ in range(N_CORES)]
    for r in range(SLOTS):
        idxs = order[r * N_CORES:(r + 1) * N_CORES]
        if r % 2 == 1:
            idxs = idxs[::-1]
        for g in range(N_CORES):
            groups[g].append(idxs[g])

    def cost(gs):
        # per-slot max after sorting each group's batches descending
        profs = [sorted((ntiles[b] for b in grp), reverse=True) for grp in gs]
        return sum(max(p[j] for p in profs) for j in range(SLOTS))

    best = cost(groups)
    improved = True
    while improved:
        improved = False
        for ga in range(N_CORES):
            for gb in range(ga + 1, N_CORES):
                for ia in range(SLOTS):
                    for ib in range(SLOTS):
                        groups[ga][ia], groups[gb][ib] = groups[gb][ib], groups[ga][ia]
                        c = cost(groups)
                        if c < best:
                            best = c
                            improved = True
                        else:
                            groups[ga][ia], groups[gb][ib] = \
                                groups[gb][ib], groups[ga][ia]
    for g in range(N_CORES):
        groups[g].sort(key=lambda b: -ntiles[b])
    slot_tiles = [max(ntiles[groups[g][j]] for g in range(N_CORES))
                  for j in range(SLOTS)]
    return groups, slot_tiles


def _prepare(decoder_feat, encoder_feat, Wq, bq, Wk, bk, Wv, bv, seq_len):
    dec = np.ascontiguousarray(decoder_feat, dtype=np.float32)
    enc = np.ascontiguousarray(encoder_feat, dtype=np.float32)
    L = np.clip(np.asarray(seq_len).astype(np.int64), 1, T)
    assert dec.shape == (B, ENC) and enc.shape == (B, T, ENC)

    groups, slot_tiles = _assign(L)
    nt_total = sum(slot_tiles)
    pad = (-nt_total) % 2  # tiles processed in pairs
    slot_tiles = list(slot_tiles)
    slot_tiles[-1] += pad
    nt_total += pad
    slot_off = np.cumsum([0] + slot_tiles)[:-1]

    bias_kv = np.concatenate([np.asarray(bk, np.float32).reshape(KQ),
                              np.asarray(bv, np.float32).reshape(VD)])
    bq = np.asarray(bq, np.float32).reshape(KQ)
    has_bias_kv = bool(np.any(bias_kv))
    has_bias_q = bool(np.any(bq))

    wkv = np.concatenate([np.asarray(Wk, np.float32),
                          np.asarray(Wv, np.float32)], axis=1)  # [ENC, 512]
    wkv16 = wkv.astype(ml_dtypes.bfloat16)

    in_maps = []
    meta = []  # per core: list of (batch, L)
    for g in range(N_CORES):
        encT = np.zeros((ENC, nt_total * PT), np.float32)
        masks = np.zeros((PT, nt_total), np.float32)
        decT = np.zeros((ENC, SLOTS), np.float32)
        core_meta = []
        for j in range(SLOTS):
            b = groups[g][j]
            l = int(L[b])
            off = int(slot_off[j]) * PT
            encT[:, off:off + l] = enc[b, :l].T
            decT[:, j] = dec[b]
            for i in range(slot_tiles[j]):
                lo = i * PT
                valid = min(max(l - lo, 0), PT)
                masks[:valid, int(slot_off[j]) + i] = 1.0
            core_meta.append((b, l))
        meta.append(core_meta)
        blobA, masks16, blob32 = pack_aux(np.asarray(Wq, np.float32), decT,
                                          masks, nt_total, has_bias_kv,
                                          has_bias_q, bias_kv, bq)
        in_maps.append({
            "encT": encT.astype(ml_dtypes.bfloat16),
            "wkv": wkv16,
            "blobA": blobA,
            "masks": masks16,
            "blob32": blob32,
        })

    key = (tuple(slot_tiles), has_bias_kv, has_bias_q)
    if key not in _CACHE:
        _CACHE[key] = _build(slot_tiles, has_bias_kv, has_bias_q)
    return _CACHE[key], in_maps, meta


def pack_aux(Wq, decT, masks, nt_total, has_bias_kv, has_bias_q, bias_kv, bq):
    """bf16 head blob: [wq(1024) | decT(32) | sel(1024)] (gates q-proj, early);
    bf16 tail blob: masks [128, nt] (needed late);
    f32 blob: [ident(128) | bias_kv2(2KV)? | bq8(KQ)?]"""
    wq4 = Wq.reshape(ECH, 128, KQ).transpose(1, 0, 2)
    sel = np.zeros((SLOTS, SLOTS * 128), np.float32)
    for j in range(SLOTS):
        sel[j, j * 128:(j + 1) * 128] = 1.0
    bw = ECH * KQ + ECH * SLOTS + SLOTS * 128
    blobA = np.zeros((128, bw), np.float32)
    blobA[:, 0:ECH * KQ] = wq4.reshape(128, ECH * KQ)
    blobA[:, ECH * KQ:ECH * KQ + ECH * SLOTS] = \
        decT.reshape(ECH, 128, SLOTS).transpose(1, 0, 2).reshape(128, ECH * SLOTS)
    blobA[0:SLOTS, ECH * KQ + ECH * SLOTS:] = sel
    fw = 128 + (2 * KV if has_bias_kv else 0) + (KQ if has_bias_q else 0)
    blob32 = np.zeros((128, fw), np.float32)
    blob32[:, 0:128] = np.eye(128, dtype=np.float32)
    fo = 128
    if has_bias_kv:
        blob32[:, fo:fo + KV] = bias_kv
        blob32[:, fo + KV:fo + 2 * KV] = bias_kv
        fo += 2 * KV
    if has_bias_q:
        blob32[0:SLOTS, fo:fo + KQ] = bq
    return (blobA.astype(ml_dtypes.bfloat16),
            masks.astype(ml_dtypes.bfloat16), blob32)


# ---------------------------------------------------------------- device

def _build(slot_tiles, has_bias_kv, has_bias_q, leaky_impl=None):
    leaky_impl = leaky_impl or LEAKY_IMPL
    nt_total = sum(slot_tiles)
    assert nt_total % 2 == 0
    slot_off = np.cumsum([0] + list(slot_tiles))[:-1]
    nc = bacc.Bacc("TRN2", target_bir_lowering=False, debug=False)

    bw = ECH * KQ + ECH * SLOTS + SLOTS * 128
    fw = 128 + (2 * KV if has_bias_kv else 0) + (KQ if has_bias_q else 0)
    encT_p = nc.declare_dram_parameter("encT", [ENC, nt_total * PT], BF16, isOutput=False)
    wkv_p = nc.declare_dram_parameter("wkv", [ENC, KV], BF16, isOutput=False)
    blobA_p = nc.declare_dram_parameter("blobA", [128, bw], BF16, isOutput=False)
    masks_p = nc.declare_dram_parameter("masks", [128, nt_total], BF16, isOutput=False)
    blob32_p = nc.declare_dram_parameter("blob32", [128, fw], F32, isOutput=False)
    score_p = nc.declare_dram_parameter("score_out", [SLOTS, T], F32, isOutput=True)
    ctx_p = nc.declare_dram_parameter("ctx_out", [SLOTS, VD], F32, isOutput=True)

    # tile g lives in pair p = g//2, half m = g%2
    def kv_slice(kv_pairs, g, lo, hi):
        return kv_pairs[g // 2][:, (g % 2) * KV + lo:(g % 2) * KV + hi]

    with tile.TileContext(nc) as tc:
        with (
            tc.tile_pool(name="const", bufs=1) as cpool,
            tc.tile_pool(name="enc", bufs=3) as epool,
            tc.tile_pool(name="kv", bufs=(nt_total + 1) // 2) as kvpool,
            tc.tile_pool(name="prod", bufs=2) as ppool,
            tc.tile_pool(name="small", bufs=2) as spool,
            tc.tile_pool(name="ps_kv", bufs=2, space="PSUM") as ps_kv,
            tc.tile_pool(name="ps_misc", bufs=1, space="PSUM") as ps_misc,
            tc.tile_pool(name="ps_ctx", bufs=1, space="PSUM") as ps_ctx,
            tc.tile_pool(name="ps_tp", bufs=1, space="PSUM") as ps_tp,
            tc.tile_pool(name="ps_tiny", bufs=1, space="PSUM") as ps_tiny,
        ):
            # ---- constants: wkv alone (gates first kv matmul), the rest in
            # two packed blobs to minimize serialized DMA-issue slices
            wkv_sb = cpool.tile([128, ECH, KV], BF16)
            nc.sync.dma_start(wkv_sb[:], wkv_p[:].rearrange("(c p) n -> p c n", p=128))
            blobA_sb = cpool.tile([128, bw], BF16)
            nc.sync.dma_start(blobA_sb[:], blobA_p[:])
            wq_sb = blobA_sb[:, 0:ECH * KQ].rearrange("p (c n) -> p c n", c=ECH)
            decT_sb = blobA_sb[:, ECH * KQ:ECH * KQ + ECH * SLOTS] \
                .rearrange("p (c n) -> p c n", c=ECH)
            sel_sb = blobA_sb[0:SLOTS, ECH * KQ + ECH * SLOTS:bw]
            blob32_sb = cpool.tile([128, fw], F32)
            masks_sb = cpool.tile([128, nt_total], BF16)

            def load_late_consts():
                nc.sync.dma_start(masks_sb[:], masks_p[:])
                nc.sync.dma_start(blob32_sb[:], blob32_p[:])

            if has_bias_kv or has_bias_q:
                load_late_consts()   # biases gate tile-0 / q-proj
                late_loaded = True
            else:
                late_loaded = False
            ident_sb = blob32_sb[:, 0:128]
            fo = 128
            if has_bias_kv:
                bias_kv2_sb = blob32_sb[:, fo:fo + 2 * KV]
                fo += 2 * KV
            if has_bias_q:
                bq8_sb = blob32_sb[0:SLOTS, fo:fo + KQ]
            ones_col = cpool.tile([128, 1], F32)
            nc.vector.memset(ones_col[:], 1.0)
            ones_row = cpool.tile([1, 128], F32)
            nc.vector.memset(ones_row[:], 1.0)

            # HAM warm-up: dummy matmuls while the first encoder chunk is in
            # flight, so real matmuls start at 2.4 GHz instead of 1.2
            warm_sb = cpool.tile([128, 128], BF16)
            nc.vector.memset(warm_sb[:], 0.0)
            warm_ps = ps_tp.tile([128, 512], F32, tag="tp", name="warmps")
            for _ in range(10):
                nc.tensor.matmul(warm_ps[:, 0:128], warm_sb[:], warm_sb[:],
                                 start=True, stop=True)

            def leaky(out_ap, in_ap, tmp_pool, tmp_shape, tmp_tag):
                if leaky_impl == "act":
                    nc.scalar.activation(out_ap, in_ap, AF.Prelu, alpha=NEG_SLOPE)
                else:
                    tmp = tmp_pool.tile(tmp_shape, F32, tag=tmp_tag)
                    nc.vector.tensor_scalar_mul(tmp[:], in_ap, NEG_SLOPE)
                    nc.vector.tensor_tensor(out_ap, in_ap, tmp[:], OP.max)

            mul_eng = nc.gpsimd if MUL_ENGINE == "gpsimd" else nc.vector

            qb_all = cpool.tile([128, SLOTS * KQ], BF16)

            def emit_qproj():
                q_ps = ps_misc.tile([SLOTS, KQ], F32, tag="qmisc")
                if has_bias_q:
                    nc.vector.tensor_copy(q_ps[:], bq8_sb)
                for c in range(ECH):
                    nc.tensor.matmul(q_ps[:], decT_sb[:, c, :], wq_sb[:, c, :],
                                     start=(c == 0 and not has_bias_q),
                                     stop=(c == ECH - 1))
                q_sb = cpool.tile([SLOTS, KQ], BF16)
                leaky(q_sb[:], q_ps[:], spool, [SLOTS, KQ], "qleak")
                for j in range(SLOTS):
                    qb_ps = ps_misc.tile([128, KQ], F32, tag="qmisc")
                    nc.tensor.matmul(qb_ps[:], sel_sb[:, j * 128:(j + 1) * 128],
                                     q_sb[:], start=True, stop=True)
                    nc.vector.tensor_copy(qb_all[:, j * KQ:(j + 1) * KQ], qb_ps[:])

            # ---- phase 2 emitter (called per slot as soon as its energies
            # are complete, so softmax/ctx overlap later slots' projections)
            ctx_all = cpool.tile([1, SLOTS * VD], F32)

            p2 = {}  # per-slot saved state between stats and out parts

            def emit_phase2(j, kv_pairs, part):
                o, n = int(slot_off[j]), slot_tiles[j]
                if part == "stats":
                    E_slot = spool.tile([128, max(slot_tiles)], BF16, tag="Eslot",
                                        name=f"E{j}")
                    nc.scalar.activation(E_slot[:, 0:n], energy_slots[j][:], AF.Exp)
                    Em = cpool.tile([128, n], BF16, name=f"Em{j}")
                    rowsum = spool.tile([128, 1], F32, tag="rowsum")
                    nc.vector.tensor_mul(Em[:], E_slot[:, 0:n],
                                         masks_sb[:, o:o + n])
                    nc.vector.tensor_reduce(rowsum[:], Em[:], AX.X, OP.add)
                    s_ps = ps_tiny.tile([1, 1], F32, tag="tiny")
                    nc.tensor.matmul(s_ps[:], rowsum[:], ones_col[:],
                                     start=True, stop=True)
                    recip = spool.tile([1, 1], F32, tag="recip", name=f"recip{j}")
                    nc.vector.reciprocal(recip[:], s_ps[:])
                    p2[j] = (Em, recip)
                    return
                Em, recip = p2.pop(j)
                bc_ps = ps_tiny.tile([128, 1], F32, tag="tiny")
                nc.tensor.matmul(bc_ps[:], ones_row[:], recip[:], start=True, stop=True)
                recip_col = spool.tile([128, 1], F32, tag="recipcol")
                nc.vector.tensor_copy(recip_col[:], bc_ps[:])

                # context: accumulate E_masked[:,g] @ v_g, then scale by 1/S
                ctx_ps = ps_ctx.tile([1, VD], F32, tag="ctxps")
                for i in range(n):
                    g = o + i
                    nc.tensor.matmul(ctx_ps[:], Em[:, i:i + 1],
                                     kv_slice(kv_pairs, g, KQ, KV),
                                     start=(i == 0), stop=(i == n - 1))
                nc.vector.tensor_scalar_mul(ctx_all[:, j * VD:(j + 1) * VD],
                                            ctx_ps[:], recip[:])

                # score: E_masked/S, transposed out to [tiles, 128]
                nw = min(n, T // PT)
                score_sb = spool.tile([128, nw], F32, tag="score")
                nc.vector.tensor_scalar_mul(score_sb[:], Em[:, 0:nw],
                                            recip_col[:])
                tp_ps = ps_tp.tile([nw, 128], F32, tag="tp")
                nc.tensor.transpose(tp_ps[:], score_sb[:], ident_sb)
                scoreT_sb = spool.tile([nw, 128], F32, tag="scoreT")
                nc.scalar.copy(scoreT_sb[:], tp_ps[:])
                nc.sync.dma_start(
                    score_p[:].rearrange("s (a b) -> s a b", b=PT)[j, 0:nw, :],
                    scoreT_sb[:])

            # ---- phase 1: K/V projections + energy products
            energy_slots = [
                cpool.tile([128, slot_tiles[jj]], F32, name=f"energy{jj}")
                for jj in range(SLOTS)]
            tile_slot = []
            for j in range(SLOTS):
                tile_slot += [j] * slot_tiles[j]
            junk = cpool.tile([128, KQ], BF16)
            kv_pairs = []
            enc_ch = None
            kv_ps = None
            # chunk schedule: small leading chunks for fast PE start
            chunk_starts = {}
            g0 = 0
            for sz in [2, 4]:
                if g0 + sz <= nt_total:
                    chunk_starts[g0] = sz
                    g0 += sz
            while g0 < nt_total:
                sz = min(CHUNK, nt_total - g0)
                chunk_starts[g0] = sz
                g0 += sz
            chunk_of = {}
            for cg0, sz in chunk_starts.items():
                for gg in range(cg0, cg0 + sz):
                    chunk_of[gg] = cg0
            pending = []   # (emit_at_tile, slot, part)
            for g in range(nt_total):
                while pending and pending[0][0] <= g:
                    _, pj, part = pending.pop(0)
                    emit_phase2(pj, kv_pairs, part)
                if g in chunk_starts:
                    nch = chunk_starts[g]
                    enc_ch = epool.tile([128, ECH, CHUNK * PT], BF16, tag="ench")
                    encT_r = encT_p[:].rearrange("(c p) t -> p c t", p=128)
                    if g == 0:
                        for c in range(ECH):
                            nc.sync.dma_start(
                                enc_ch[:, c, 0:nch * PT],
                                encT_r[:, c, g * PT:(g + nch) * PT])
                    else:
                        nc.sync.dma_start(
                            enc_ch[:, :, 0:nch * PT],
                            encT_r[:, :, g * PT:(g + nch) * PT])
                t0 = (g - chunk_of[g]) * PT
                if g % 2 == 0:
                    kv_ps = ps_kv.tile([128, 2 * KV], F32, tag="kvps")
                    if has_bias_kv:
                        nc.vector.tensor_copy(kv_ps[:], bias_kv2_sb)
                half = (g % 2) * KV
                for c in range(ECH):
                    nc.tensor.matmul(kv_ps[:, half:half + KV],
                                     enc_ch[:, c, t0:t0 + PT],
                                     wkv_sb[:, c, :],
                                     start=(c == 0 and not has_bias_kv),
                                     stop=(c == ECH - 1))
                if g == 1:
                    emit_qproj()
                if g == 3 and not late_loaded:
                    load_late_consts()
                if g % 2 == 0:
                    continue
                # pair (g-1, g) complete: leaky, then fused energy mul+reduce
                kv_sb = kvpool.tile([128, 2 * KV], BF16, tag="kv")
                leaky(kv_sb[:], kv_ps[:], spool, [128, 2 * KV], "kvleak")
                kv_pairs.append(kv_sb)
                for gg in (g - 1, g):
                    j = tile_slot[gg]
                    i = gg - int(slot_off[j])
                    nc.vector.affine_mul_reduce(
                        junk[:], energy_slots[j][:, i:i + 1],
                        kv_slice(kv_pairs, gg, 0, KQ),
                        qb_all[:, j * KQ:(j + 1) * KQ],
                        scale=1.0, bias=0.0)
                    if i == slot_tiles[j] - 1:
                        pending.append((g + 5, j, "stats"))
                        pending.append((g + 8, j, "out"))

            for _, pj, part in pending:
                emit_phase2(pj, kv_pairs, part)
            nc.sync.dma_start(ctx_p[:].rearrange("s v -> (s v)")[None, :], ctx_all[:])

    nc.compile()
    return nc


# ---------------------------------------------------------------- entry

def kernel(decoder_feat, encoder_feat, Wq, bq, Wk, bk, Wv, bv, seq_len):
    from concourse.bass_utils import run_bass_kernel_spmd

    nc, in_maps, meta = _prepare(decoder_feat, encoder_feat, Wq, bq,
                                 Wk, bk, Wv, bv, seq_len)
    res = run_bass_kernel_spmd(nc, in_maps, list(range(N_CORES)))
    return _gather(res.results, meta)


def _gather(results, meta):
    score = np.zeros((B, T), np.float32)
    ctx = np.zeros((B, VD), np.float32)
    for g in range(N_CORES):
        sc = np.asarray(results[g]["score_out"], np.float32)
        cx = np.asarray(results[g]["ctx_out"], np.float32)
        for j, (b, l) in enumerate(meta[g]):
            score[b, :l] = sc[j, :l]
            ctx[b] = cx[j]
    return score, ctx


# revision 49
# speedup vs baseline: 1.2335x; 1.0621x over previous
"""Trainium2 Bass kernel for single-query ragged attention.

reference math (per batch b, L = seq_len[b]):
  q = leaky(dec @ Wq + bq)                   [KQ]
  k = leaky(enc[:L] @ Wk + bk)               [L, KQ]
  v = leaky(enc[:L] @ Wv + bv)               [L, VD]
  e = k @ q                                  [L]
  s = softmax(e) (masked to L, renormalized) [T] (zeros beyond L)
  ctx = s[:L] @ v                            [VD]

Strategy: data-parallel over batch across 8 NeuronCores, 8 batch "slots"
per core. Ragged: only ceil(L/128) row-tiles per batch are computed; the
kernel is compile-time specialized to the actual seq_len profile (slot j
runs max-over-cores tiles; shorter batches are zero-padded and masked).
Host pre-transposes encoder shards to [ENC, L] bf16 so the contraction
dim lands on SBUF partitions. Compute: bf16 matmuls w/ f32 PSUM
accumulation, f32 softmax statistics.

Engine budget per tile (128 t-rows): PE 4x MM[128x128x512] (~213ns ea),
ACT 1/2 Prelu[128,1024] (~500ns), GpSimd product mul[128,256],
DVE per-slot batched reduce. Energy/softmax stats in f32.
"""

import math

import ml_dtypes
import numpy as np

import concourse.bass as bass
import concourse.tile as tile
from concourse import bacc, mybir

B, T, ENC = 64, 2048, 512
KQ, VD = 256, 256
KV = KQ + VD
NEG_SLOPE = 0.2
N_CORES = 8
SLOTS = 8          # batches per core
PT = 128           # partition tile (rows of t per tile)
ECH = ENC // 128   # 4 contraction chunks
CHUNK = 8          # t-tiles per DMA

F32 = mybir.dt.float32
BF16 = mybir.dt.bfloat16
AF = mybir.ActivationFunctionType
OP = mybir.AluOpType
AX = mybir.AxisListType

LEAKY_IMPL = "act"       # "act" = ScalarE Prelu; "dve" = VectorE mul+max
MUL_ENGINE = "gpsimd"    # engine for energy product mul: "gpsimd" | "vector"

_CACHE = {}


# ---------------------------------------------------------------- host prep

def _assign(seq_len):
    """Distribute 64 batches into 8 cores x 8 slots minimizing the padded
    tile total sum_j max_g(ntiles). Snake seed + swap refinement."""
    ntiles = [max(1, math.ceil(int(l) / PT)) for l in seq_len]
    order = sorted(range(B), key=lambda b: -ntiles[b])
    groups = [[] for _ in range(N_CORES)]
    for r in range(SLOTS):
        idxs = order[r * N_CORES:(r + 1) * N_CORES]
        if r % 2 == 1:
            idxs = idxs[::-1]
        for g in range(N_CORES):
            groups[g].append(idxs[g])

    def cost(gs):
        # per-slot max after sorting each group's batches descending
        profs = [sorted((ntiles[b] for b in grp), reverse=True) for grp in gs]
        return sum(max(p[j] for p in profs) for j in range(SLOTS))

    best = cost(groups)
    improved = True
    while improved:
        improved = False
        for ga in range(N_CORES):
            for gb in range(ga + 1, N_CORES):
                for ia in range(SLOTS):
                    for ib in range(SLOTS):
                        groups[ga][ia], groups[gb][ib] = groups[gb][ib], groups[ga][ia]
                        c = cost(groups)
                        if c < best:
                            best = c
                            improved = True
                        else:
                            groups[ga][ia], groups[gb][ib] = \
                                groups[gb][ib], groups[ga][ia]
    for g in range(N_CORES):
        groups[g].sort(key=lambda b: -ntiles[b])
    slot_tiles = [max(ntiles[groups[g][j]] for g in range(N_CORES))
                  for j in range(SLOTS)]
    return groups, slot_tiles


def _prepare(decoder_feat, encoder_feat, Wq, bq, Wk, bk, Wv, bv, seq_len):
    dec = np.ascontiguousarray(decoder_feat, dtype=np.float32)
    enc = np.ascontiguousarray(encoder_feat, dtype=np.float32)
    L = np.clip(np.asarray(seq_len).astype(np.int64), 1, T)
    assert dec.shape == (B, ENC) and enc.shape == (B, T, ENC)

    groups, slot_tiles = _assign(L)
    nt_total = sum(slot_tiles)
    pad = (-nt_total) % 2  # tiles processed in pairs
    slot_tiles = list(slot_tiles)
    slot_tiles[-1] += pad
    nt_total += pad
    slot_off = np.cumsum([0] + slot_tiles)[:-1]

    bias_kv = np.concatenate([np.asarray(bk, np.float32).reshape(KQ),
                              np.asarray(bv, np.float32).reshape(VD)])
    bq = np.asarray(bq, np.float32).reshape(KQ)
    has_bias_kv = bool(np.any(bias_kv))
    has_bias_q = bool(np.any(bq))

    wkv = np.concatenate([np.asarray(Wk, np.float32),
                          np.asarray(Wv, np.float32)], axis=1)  # [ENC, 512]
    wkv16 = wkv.astype(ml_dtypes.bfloat16)

    in_maps = []
    meta = []  # per core: list of (batch, L)
    for g in range(N_CORES):
        encT = np.zeros((ENC, nt_total * PT), np.float32)
        masks = np.zeros((PT, nt_total), np.float32)
        decT = np.zeros((ENC, SLOTS), np.float32)
        core_meta = []
        for j in range(SLOTS):
            b = groups[g][j]
            l = int(L[b])
            off = int(slot_off[j]) * PT
            encT[:, off:off + l] = enc[b, :l].T
            decT[:, j] = dec[b]
            for i in range(slot_tiles[j]):
                lo = i * PT
                valid = min(max(l - lo, 0), PT)
                masks[:valid, int(slot_off[j]) + i] = 1.0
            core_meta.append((b, l))
        meta.append(core_meta)
        blobA, masks16, blob32 = pack_aux(np.asarray(Wq, np.float32), decT,
                                          masks, nt_total, has_bias_kv,
                                          has_bias_q, bias_kv, bq)
        in_maps.append({
            "encT": encT.astype(ml_dtypes.bfloat16),
            "wkv": wkv16,
            "blobA": blobA,
            "masks": masks16,
            "blob32": blob32,
        })

    key = (tuple(slot_tiles), has_bias_kv, has_bias_q)
    if key not in _CACHE:
        _CACHE[key] = _build(slot_tiles, has_bias_kv, has_bias_q)
    return _CACHE[key], in_maps, meta


def pack_aux(Wq, decT, masks, nt_total, has_bias_kv, has_bias_q, bias_kv, bq):
    """bf16 head blob: [wq(1024) | decT(32) | sel(1024)] (gates q-proj, early);
    bf16 tail blob: masks [128, nt] (needed late);
    f32 blob: [ident(128) | bias_kv2(2KV)? | bq8(KQ)?]"""
    wq4 = Wq.reshape(ECH, 128, KQ).transpose(1, 0, 2)
    sel = np.zeros((SLOTS, SLOTS * 128), np.float32)
    for j in range(SLOTS):
        sel[j, j * 128:(j + 1) * 128] = 1.0
    bw = ECH * KQ + ECH * SLOTS + SLOTS * 128
    blobA = np.zeros((128, bw), np.float32)
    blobA[:, 0:ECH * KQ] = wq4.reshape(128, ECH * KQ)
    blobA[:, ECH * KQ:ECH * KQ + ECH * SLOTS] = \
        decT.reshape(ECH, 128, SLOTS).transpose(1, 0, 2).reshape(128, ECH * SLOTS)
    blobA[0:SLOTS, ECH * KQ + ECH * SLOTS:] = sel
    fw = 128 + (2 * KV if has_bias_kv else 0) + (KQ if has_bias_q else 0)
    blob32 = np.zeros((128, fw), np.float32)
    blob32[:, 0:128] = np.eye(128, dtype=np.float32)
    fo = 128
    if has_bias_kv:
        blob32[:, fo:fo + KV] = bias_kv
        blob32[:, fo + KV:fo + 2 * KV] = bias_kv
        fo += 2 * KV
    if has_bias_q:
        blob32[0:SLOTS, fo:fo + KQ] = bq
    return (blobA.astype(ml_dtypes.bfloat16),
            masks.astype(ml_dtypes.bfloat16), blob32)


# ---------------------------------------------------------------- device

def _build(slot_tiles, has_bias_kv, has_bias_q, leaky_impl=None):
    leaky_impl = leaky_impl or LEAKY_IMPL
    nt_total = sum(slot_tiles)
    assert nt_total % 2 == 0
    slot_off = np.cumsum([0] + list(slot_tiles))[:-1]
    nc = bacc.Bacc("TRN2", target_bir_lowering=False, debug=False)

    bw = ECH * KQ + ECH * SLOTS + SLOTS * 128
    fw = 128 + (2 * KV if has_bias_kv else 0) + (KQ if has_bias_q else 0)
    encT_p = nc.declare_dram_parameter("encT", [ENC, nt_total * PT], BF16, isOutput=False)
    wkv_p = nc.declare_dram_parameter("wkv", [ENC, KV], BF16, isOutput=False)
    blobA_p = nc.declare_dram_parameter("blobA", [128, bw], BF16, isOutput=False)
    masks_p = nc.declare_dram_parameter("masks", [128, nt_total], BF16, isOutput=False)
    blob32_p = nc.declare_dram_parameter("blob32", [128, fw], F32, isOutput=False)
    score_p = nc.declare_dram_parameter("score_out", [SLOTS, T], F32, isOutput=True)
    ctx_p = nc.declare_dram_parameter("ctx_out", [SLOTS, VD], F32, isOutput=True)

    # tile g lives in pair p = g//2, half m = g%2
    def kv_slice(kv_pairs, g, lo, hi):
        return kv_pairs[g // 2][:, (g % 2) * KV + lo:(g % 2) * KV + hi]

    with tile.TileContext(nc) as tc:
        with (
            tc.tile_pool(name="const", bufs=1) as cpool,
            tc.tile_pool(name="enc", bufs=3) as epool,
            tc.tile_pool(name="kv", bufs=(nt_total + 1) // 2) as kvpool,
            tc.tile_pool(name="prod", bufs=2) as ppool,
            tc.tile_pool(name="small", bufs=2) as spool,
            tc.tile_pool(name="ps_kv", bufs=2, space="PSUM") as ps_kv,
            tc.tile_pool(name="ps_misc", bufs=1, space="PSUM") as ps_misc,
            tc.tile_pool(name="ps_ctx", bufs=1, space="PSUM") as ps_ctx,
            tc.tile_pool(name="ps_tp", bufs=1, space="PSUM") as ps_tp,
            tc.tile_pool(name="ps_tiny", bufs=1, space="PSUM") as ps_tiny,
        ):
            # ---- constants: wkv alone (gates first kv matmul), the rest in
            # two packed blobs to minimize serialized DMA-issue slices
            wkv_sb = cpool.tile([128, ECH, KV], BF16)
            nc.sync.dma_start(wkv_sb[:], wkv_p[:].rearrange("(c p) n -> p c n", p=128))
            blobA_sb = cpool.tile([128, bw], BF16)
            nc.sync.dma_start(blobA_sb[:], blobA_p[:])
            wq_sb = blobA_sb[:, 0:ECH * KQ].rearrange("p (c n) -> p c n", c=ECH)
            decT_sb = blobA_sb[:, ECH * KQ:ECH * KQ + ECH * SLOTS] \
                .rearrange("p (c n) -> p c n", c=ECH)
            sel_sb = blobA_sb[0:SLOTS, ECH * KQ + ECH * SLOTS:bw]
            blob32_sb = cpool.tile([128, fw], F32)
            masks_sb = cpool.tile([128, nt_total], BF16)

            def load_late_consts():
                nc.sync.dma_start(masks_sb[:], masks_p[:])
                nc.sync.dma_start(blob32_sb[:], blob32_p[:])

            if has_bias_kv or has_bias_q:
                load_late_consts()   # biases gate tile-0 / q-proj
                late_loaded = True
            else:
                late_loaded = False
            ident_sb = blob32_sb[:, 0:128]
            fo = 128
            if has_bias_kv:
                bias_kv2_sb = blob32_sb[:, fo:fo + 2 * KV]
                fo += 2 * KV
            if has_bias_q:
                bq8_sb = blob32_sb[0:SLOTS, fo:fo + KQ]
            ones128 = cpool.tile([128, 128], F32)
            nc.vector.memset(ones128[:], 1.0)

            # HAM warm-up: dummy matmuls while the first encoder chunk is in
            # flight, so real matmuls start at 2.4 GHz instead of 1.2
            warm_sb = cpool.tile([128, 512], BF16)
            nc.vector.memset(warm_sb[:], 0.0)
            warm_ps = ps_tp.tile([128, 512], F32, tag="tp", name="warmps")
            for _ in range(11):
                nc.tensor.matmul(warm_ps[:], warm_sb[:, 0:128], warm_sb[:],
                                 start=True, stop=True)

            def leaky(out_ap, in_ap, tmp_pool, tmp_shape, tmp_tag):
                if leaky_impl == "act":
                    nc.scalar.activation(out_ap, in_ap, AF.Prelu, alpha=NEG_SLOPE)
                else:
                    tmp = tmp_pool.tile(tmp_shape, F32, tag=tmp_tag)
                    nc.vector.tensor_scalar_mul(tmp[:], in_ap, NEG_SLOPE)
                    nc.vector.tensor_tensor(out_ap, in_ap, tmp[:], OP.max)

            mul_eng = nc.gpsimd if MUL_ENGINE == "gpsimd" else nc.vector

            qb_all = cpool.tile([128, SLOTS * KQ], BF16)

            def emit_qproj():
                q_ps = ps_misc.tile([SLOTS, KQ], F32, tag="qmisc")
                if has_bias_q:
                    nc.vector.tensor_copy(q_ps[:], bq8_sb)
                for c in range(ECH):
                    nc.tensor.matmul(q_ps[:], decT_sb[:, c, :], wq_sb[:, c, :],
                                     start=(c == 0 and not has_bias_q),
                                     stop=(c == ECH - 1))
                q_sb = cpool.tile([SLOTS, KQ], BF16)
                leaky(q_sb[:], q_ps[:], spool, [SLOTS, KQ], "qleak")
                for j in range(SLOTS):
                    qb_ps = ps_misc.tile([128, KQ], F32, tag="qmisc")
                    nc.tensor.matmul(qb_ps[:], sel_sb[:, j * 128:(j + 1) * 128],
                                     q_sb[:], start=True, stop=True)
                    nc.vector.tensor_copy(qb_all[:, j * KQ:(j + 1) * KQ], qb_ps[:])

            # ---- phase 2 emitter (called per slot as soon as its energies
            # are complete, so softmax/ctx overlap later slots' projections)
            ctx_all = cpool.tile([1, SLOTS * VD], F32)

            p2 = {}  # per-slot saved state between stats and out parts

            def emit_phase2(j, kv_pairs, part):
                o, n = int(slot_off[j]), slot_tiles[j]
                if part == "stats":
                    E_slot = spool.tile([128, max(slot_tiles)], BF16, tag="Eslot",
                                        name=f"E{j}")
                    nc.scalar.activation(E_slot[:, 0:n], energy_slots[j][:], AF.Exp)
                    Em = cpool.tile([128, n], BF16, name=f"Em{j}")
                    rowsum = spool.tile([128, 1], F32, tag="rowsum")
                    nc.vector.tensor_mul(Em[:], E_slot[:, 0:n],
                                         masks_sb[:, o:o + n])
                    nc.vector.tensor_reduce(rowsum[:], Em[:], AX.X, OP.add)
                    # total-on-all-partitions: ones[128,128] @ rowsum
                    s_ps = ps_tiny.tile([128, 1], F32, tag="tiny")
                    nc.tensor.matmul(s_ps[:], ones128[:], rowsum[:],
                                     start=True, stop=True)
                    recip_col = spool.tile([128, 1], F32, tag="recipcol",
                                           name=f"recipcol{j}")
                    nc.vector.reciprocal(recip_col[:], s_ps[:])
                    p2[j] = (Em, recip_col)
                    return
                Em, recip_col = p2.pop(j)

                # context: accumulate E_masked[:,g] @ v_g, then scale by 1/S
                ctx_ps = ps_ctx.tile([1, VD], F32, tag="ctxps")
                for i in range(n):
                    g = o + i
                    nc.tensor.matmul(ctx_ps[:], Em[:, i:i + 1],
                                     kv_slice(kv_pairs, g, KQ, KV),
                                     start=(i == 0), stop=(i == n - 1))
                nc.vector.tensor_scalar_mul(ctx_all[:, j * VD:(j + 1) * VD],
                                            ctx_ps[:], recip_col[0:1, :])
                nc.gpsimd.dma_start(
                    ctx_p[:].rearrange("s v -> (s v)")[None, j * VD:(j + 1) * VD],
                    ctx_all[:, j * VD:(j + 1) * VD])

                # score: E_masked/S, written column-major [128, tiles];
                # the host gather undoes the layout (t = i*128 + p)
                nw = min(n, T // PT)
                score_sb = spool.tile([128, nw], F32, tag="score")
                nc.vector.tensor_scalar_mul(score_sb[:], Em[:, 0:nw],
                                            recip_col[:])
                nc.sync.dma_start(
                    score_p[:].rearrange("s (p i) -> s p i", i=T // PT)
                    [j, :, 0:nw], score_sb[:])

            # ---- phase 1: K/V projections + energy products
            energy_slots = [
                cpool.tile([128, slot_tiles[jj]], F32, name=f"energy{jj}")
                for jj in range(SLOTS)]
            tile_slot = []
            for j in range(SLOTS):
                tile_slot += [j] * slot_tiles[j]
            junk = cpool.tile([128, KQ], BF16)
            kv_pairs = []
            enc_ch = None
            kv_ps = None
            # chunk schedule: small leading chunks for fast PE start
            chunk_starts = {}
            g0 = 0
            for sz in [2, 2, 4]:
                if g0 + sz <= nt_total:
                    chunk_starts[g0] = sz
                    g0 += sz
            while g0 < nt_total:
                sz = min(CHUNK, nt_total - g0)
                chunk_starts[g0] = sz
                g0 += sz
            chunk_of = {}
            for cg0, sz in chunk_starts.items():
                for gg in range(cg0, cg0 + sz):
                    chunk_of[gg] = cg0
            pending = []   # (emit_at_tile, slot, part)
            for g in range(nt_total):
                while pending and pending[0][0] <= g:
                    _, pj, part = pending.pop(0)
                    emit_phase2(pj, kv_pairs, part)
                if g in chunk_starts:
                    nch = chunk_starts[g]
                    enc_ch = epool.tile([128, ECH, CHUNK * PT], BF16, tag="ench")
                    encT_r = encT_p[:].rearrange("(c p) t -> p c t", p=128)
                    if g == 0:
                        for c in range(ECH):
                            nc.sync.dma_start(
                                enc_ch[:, c, 0:nch * PT],
                                encT_r[:, c, g * PT:(g + nch) * PT])
                    else:
                        nc.sync.dma_start(
                            enc_ch[:, :, 0:nch * PT],
                            encT_r[:, :, g * PT:(g + nch) * PT])
                t0 = (g - chunk_of[g]) * PT
                if g % 2 == 0:
                    kv_ps = ps_kv.tile([128, 2 * KV], F32, tag="kvps")
                    if has_bias_kv:
                        nc.vector.tensor_copy(kv_ps[:], bias_kv2_sb)
                half = (g % 2) * KV
                for c in range(ECH):
                    nc.tensor.matmul(kv_ps[:, half:half + KV],
                                     enc_ch[:, c, t0:t0 + PT],
                                     wkv_sb[:, c, :],
                                     start=(c == 0 and not has_bias_kv),
                                     stop=(c == ECH - 1))
                if g == 1:
                    emit_qproj()
                if g == 3 and not late_loaded:
                    load_late_consts()
                if g % 2 == 0:
                    continue
                # pair (g-1, g) complete: leaky, then fused energy mul+reduce
                kv_sb = kvpool.tile([128, 2 * KV], BF16, tag="kv")
                leaky(kv_sb[:], kv_ps[:], spool, [128, 2 * KV], "kvleak")
                kv_pairs.append(kv_sb)
                for gg in (g - 1, g):
                    j = tile_slot[gg]
                    i = gg - int(slot_off[j])
                    nc.vector.affine_mul_reduce(
                        junk[:], energy_slots[j][:, i:i + 1],
                        kv_slice(kv_pairs, gg, 0, KQ),
                        qb_all[:, j * KQ:(j + 1) * KQ],
                        scale=1.0, bias=0.0)
                    if i == slot_tiles[j] - 1:
                        pending.append((g + 5, j, "stats"))
                        pending.append((g + 8, j, "out"))

            for _, pj, part in sorted(pending, key=lambda x: x[2] != "stats"):
                emit_phase2(pj, kv_pairs, part)

    nc.compile()
    return nc


# ---------------------------------------------------------------- entry

def kernel(decoder_feat, encoder_feat, Wq, bq, Wk, bk, Wv, bv, seq_len):
    from concourse.bass_utils import run_bass_kernel_spmd

    nc, in_maps, meta = _prepare(decoder_feat, encoder_feat, Wq, bq,
                                 Wk, bk, Wv, bv, seq_len)
    res = run_bass_kernel_spmd(nc, in_maps, list(range(N_CORES)))
    return _gather(res.results, meta)


def _gather(results, meta):
    score = np.zeros((B, T), np.float32)
    ctx = np.zeros((B, VD), np.float32)
    for g in range(N_CORES):
        sc = np.asarray(results[g]["score_out"], np.float32)
        cx = np.asarray(results[g]["ctx_out"], np.float32)
        for j, (b, l) in enumerate(meta[g]):
            row = sc[j].reshape(PT, T // PT).T.reshape(T)
            score[b, :l] = row[:l]
            ctx[b] = cx[j]
    return score, ctx
